# revision 11
# baseline (speedup 1.0000x reference)
"""Trainium2 Bass kernel for nn_DistanceProbeAlternative (retrieval_knn).

Computes, per batch b:
    proj = emb[b] @ W.T                      # [S, R]
    dist[i, j] = ||proj_i||^2 - 2 proj_i . proj_j + ||proj_j||^2

Sharding: data-parallel over batch B=32 across 8 cores (4 batches/core).
W is replicated. No collectives.

Per-core dataflow (all matmuls in float32r for full-rate PE):
  1. DMA emb s-tiles in ([128, 2048] chunks, 1 MiB each).
  2. PE-transpose 128x128 blocks of emb -> PSUM -> copy to SBUF embT [d, s].
  3. projT[r, s] = sum_k WT_k.T @ embT_k  (PSUM accumulate over 8 d-tiles).
  4. sq = projT^2 (DVE); norms via ones-matmuls (col [128,1] per i-tile and
     row [1, S]).
  5. dots i-tile = projT_i.T @ projT  plus a K=1 matmul accumulating
     ones x (-0.5 * norms_row) into the same PSUM bank.
  6. Epilogue (single pass): out = -2 * psum + norms_col  via ACT Identity
     (scale/bias) or DVE tensor_scalar, alternating engines.
  7. DMA out [128, 2048] chunks (1 MiB each).
"""

import numpy as np
from contextlib import ExitStack

import concourse.bass as bass
import concourse.bacc as bacc
import concourse.tile as tile
from concourse import mybir
from concourse.bass_utils import run_bass_kernel_spmd
from concourse.masks import make_identity

B, S, D, R = 32, 1024, 1024, 128
NCORES = 8
BPC = B // NCORES  # batches per core

F32 = mybir.dt.float32
F32R = mybir.dt.float32r
IDENT = mybir.ActivationFunctionType.Identity


def build_nc():
    nc = bacc.Bacc("TRN2", target_bir_lowering=False, debug=False)

    emb = nc.dram_tensor("embeddings_batch", [BPC, S, D], F32, kind="ExternalInput")
    Wd = nc.dram_tensor("W", [R, D], F32, kind="ExternalInput")
    out = nc.dram_tensor("out", [BPC, S, S], F32, kind="ExternalOutput")

    NST = S // 128  # 8 s-tiles per batch
    NDT = D // 128  # 8 d-tiles

    with tile.TileContext(nc) as tc, ExitStack() as ctx:
        constp = ctx.enter_context(tc.tile_pool(name="const", bufs=1))
        embin_p = ctx.enter_context(tc.tile_pool(name="embin", bufs=4))
        embT_p = ctx.enter_context(tc.tile_pool(name="embT", bufs=2))
        projT_p = ctx.enter_context(tc.tile_pool(name="projT", bufs=2))
        sq_p = ctx.enter_context(tc.tile_pool(name="sq", bufs=2))
        ncol_p = ctx.enter_context(tc.tile_pool(name="ncol", bufs=2))
        nrow_p = ctx.enter_context(tc.tile_pool(name="nrow", bufs=2))
        out_p = ctx.enter_context(tc.tile_pool(name="outsb", bufs=3))
        tpsum_p = ctx.enter_context(tc.tile_pool(name="tpsum", bufs=2, space="PSUM"))
        projps_p = ctx.enter_context(tc.tile_pool(name="projps", bufs=1, space="PSUM"))
        dotps_p = ctx.enter_context(tc.tile_pool(name="dotps", bufs=4, space="PSUM"))

        identity = constp.tile([128, 128], F32, name="identity")
        make_identity(nc, identity)
        onesf = constp.tile([128, 128], F32, name="onesf")
        nc.gpsimd.memset(onesf, 1.0)
        ones = constp.tile([128, 128], F32R, name="ones")
        nc.vector.tensor_copy(ones, onesf)

        Wsb = constp.tile([128, D], F32, name="Wsb")
        nc.sync.dma_start(out=Wsb, in_=Wd.ap())

        # WT_k = W[:, 128k:128(k+1)].T stored at WTsb[:, 128k:128(k+1)]
        WTsb = constp.tile([128, D], F32R, name="WTsb")
        for g in range(NDT // 4):
            wtp = tpsum_p.tile([128, 512], F32, tag="tp", name="wtp")
            for j in range(4):
                k = g * 4 + j
                nc.tensor.transpose(
                    wtp[:, 128 * j : 128 * (j + 1)],
                    Wsb[:, 128 * k : 128 * (k + 1)],
                    identity,
                )
            nc.vector.tensor_copy(WTsb[:, 512 * g : 512 * (g + 1)], wtp)

        for b in range(BPC):
            # ---- load emb + transpose into embT [d, s] ----
            embT = embT_p.tile([128, NDT * S], F32R, name="embT")
            embT3 = embT.rearrange("p (k s) -> p k s", k=NDT)
            for q in range(4):  # quarter-batch: 2 s-tiles
                esb = embin_p.tile([128, 2048], F32, name="esb")
                src = emb.ap()[b, 256 * q : 256 * (q + 1), :].rearrange(
                    "(t p) d -> p t d", p=128
                )
                nc.sync.dma_start(out=esb.rearrange("p (t d) -> p t d", t=2), in_=src)
                for t in range(2):
                    i = 2 * q + t  # s-tile index
                    for g in range(2):  # group of 4 d-chunks
                        tp = tpsum_p.tile([128, 512], F32, tag="tp", name="tp")
                        for j in range(4):
                            k = g * 4 + j
                            nc.tensor.transpose(
                                tp[:, 128 * j : 128 * (j + 1)],
                                esb[:, 1024 * t + 128 * k : 1024 * t + 128 * (k + 1)],
                                identity,
                            )
                        dst = embT3[:, g * 4 : g * 4 + 4, 128 * i : 128 * (i + 1)]
                        tp4 = tp.rearrange("p (k s) -> p k s", k=4)
                        if (i + g) % 2 == 0:
                            nc.vector.tensor_copy(dst, tp4)
                        else:
                            nc.scalar.copy(dst, tp4)

            # ---- projT[r, s] accumulation over d-tiles ----
            projps = projps_p.tile([128, S], F32, name="projps")
            for k in range(NDT):
                for h in range(2):
                    nc.tensor.matmul(
                        projps[:, 512 * h : 512 * (h + 1)],
                        WTsb[:, 128 * k : 128 * (k + 1)],
                        embT[:, S * k + 512 * h : S * k + 512 * (h + 1)],
                        start=(k == 0),
                        stop=(k == NDT - 1),
                    )
            projT = projT_p.tile([128, S], F32R, name="projT")
            nc.vector.tensor_copy(projT, projps)

            # ---- norms ----
            sq = sq_p.tile([128, S], F32R, name="sq")
            nc.vector.tensor_mul(sq, projT, projT)

            # N=2 (ones columns) keeps the fp32r even-count/8B-alignment rules
            ncol_ps = tpsum_p.tile([128, 512], F32, tag="tp", name="ncol_ps")
            for i in range(NST):
                nc.tensor.matmul(
                    ncol_ps[:, 2 * i : 2 * i + 2],
                    sq[:, 128 * i : 128 * (i + 1)],
                    ones[:, 0:2],
                    start=True,
                    stop=True,
                )
            ncol = ncol_p.tile([128, 2 * NST], F32, name="ncol")
            nc.scalar.copy(ncol, ncol_ps[:, 0 : 2 * NST])

            nrow = nrow_p.tile([1, S], F32R, name="nrow")
            for h in range(2):
                nr_ps = tpsum_p.tile([1, 512], F32, tag="tp", name="nr_ps")
                nc.tensor.matmul(
                    nr_ps,
                    ones[:, 0:1],
                    sq[:, 512 * h : 512 * (h + 1)],
                    start=True,
                    stop=True,
                )
                # nrow holds -0.5 * norms_row
                nc.scalar.activation(
                    nrow[0:1, 512 * h : 512 * (h + 1)], nr_ps, IDENT, bias=0.0,
                    scale=-0.5,
                )

            # ---- dots + epilogue, two i-tiles at a time ----
            for pair in range(NST // 2):
                outsb = out_p.tile([128, 2048], F32, name="outsb")
                dps = [None] * 4
                for t in range(2):
                    i = 2 * pair + t
                    for h in range(2):
                        d_ps = dotps_p.tile([128, 512], F32, tag="dp", name="d_ps")
                        dps[2 * t + h] = d_ps
                        nc.tensor.matmul(
                            d_ps,
                            projT[:, 128 * i : 128 * (i + 1)],
                            projT[:, 512 * h : 512 * (h + 1)],
                            start=True,
                            stop=False,
                        )
                # K=1 pass: add ones x (-0.5*norms_row); single lhsT load
                for t in range(2):
                    for h in range(2):
                        nc.tensor.matmul(
                            dps[2 * t + h],
                            ones[0:1, 0:128],
                            nrow[0:1, 512 * h : 512 * (h + 1)],
                            start=False,
                            stop=True,
                        )
                for t in range(2):
                    i = 2 * pair + t
                    for h in range(2):
                        dst = outsb[:, 1024 * t + 512 * h : 1024 * t + 512 * (h + 1)]
                        if (t + h) % 2 == 0:
                            nc.scalar.activation(
                                dst, dps[2 * t + h], IDENT,
                                bias=ncol[:, 2 * i : 2 * i + 1], scale=-2.0,
                            )
                        else:
                            nc.vector.tensor_scalar(
                                dst, dps[2 * t + h], -2.0, ncol[:, 2 * i : 2 * i + 1],
                                mybir.AluOpType.mult, mybir.AluOpType.add,
                            )
                dram_dst = out.ap()[b, 256 * pair : 256 * (pair + 1), :].rearrange(
                    "(t p) d -> p t d", p=128
                )
                nc.sync.dma_start(
                    out=dram_dst, in_=outsb.rearrange("p (t d) -> p t d", t=2)
                )

    nc.finalize()
    return nc


_NC_CACHE = None


def _get_nc():
    global _NC_CACHE
    if _NC_CACHE is None:
        _NC_CACHE = build_nc()
    return _NC_CACHE


def run(embeddings_batch, W, trace=False, tmpdir=None):
    nc = _get_nc()
    emb = np.asarray(embeddings_batch, dtype=np.float32)
    Wf = np.ascontiguousarray(np.asarray(W, dtype=np.float32))
    in_maps = [
        {
            "embeddings_batch": np.ascontiguousarray(emb[c * BPC : (c + 1) * BPC]),
            "W": Wf,
        }
        for c in range(NCORES)
    ]
    res = run_bass_kernel_spmd(
        nc, in_maps, core_ids=list(range(NCORES)), trace=trace, tmpdir=tmpdir
    )
    full = np.concatenate([r["out"] for r in res.results], axis=0)
    return full, res


def kernel(embeddings_batch, W):
    full, _ = run(embeddings_batch, W, trace=False)
    return full
